# revision 31
# baseline (speedup 1.0000x reference)
"""GRU cell on 8 Trainium2 NeuronCores.

Reference computation (B=65536, D=256):
    z = sigmoid(x@Wz + h@Uz + bz)
    r = sigmoid(x@Wr + h@Ur + br)
    h_hat = tanh(x@Wh + (r*h)@Uh + bh)
    h_t = z*h + (1-z)*h_hat  ; returns (h_t, h_t)

Strategy: data-parallel over the batch dim (8 shards of 8192 rows), all
fp16 on chip (rel_l2 ~1.1e-3 vs the f32 reference; gate is 2e-2).
Measured ~103.5-105us HW exec vs the 124.5us fp32r baseline; the matmul
stream itself runs at the N=512 issue floor (216ns/matmul, 384 matmuls
= 83us), so the remaining time is the fixed framework preamble, the
initial HBM fill, and the drain tail.  Key structure:
  * host packs each shard as [128 partitions, 4 blocks, 8192] fp16
    (blocks = x k0, x k1, h k0, h k1): the contraction dim of all six
    GEMMs is the SBUF partition dim, fp16 halves HBM traffic and
    streams the PE at full rate with fast weight load,
  * all input tiles are SBUF-resident (8.4MB of 24MB), loaded with a
    carefully ordered trigger schedule: only ~4 DMA-completion
    semaphores exist per trigger engine and a trigger past the 4th
    blocks its engine queue until an earlier DMA completes, so the
    first four slots of SyncE/ScalarE carry exactly the critical set
    (r-gate weights + j=0 x tiles), h tiles ride GpSimd, and the bulk
    stream self-paces on SyncE's semaphore rotation,
  * each gate's four matmuls run W(x-side) for both output halves
    first, U(h-side) after - matching DMA arrival order and giving the
    r*h producer slack,
  * the r-gate of work item i+1 is computed one iteration early so its
    sigmoid + r*h (ScalarE+VectorE) never gate the candidate matmuls,
  * dummy warm-up matmuls during the head DMAs hold the PE's HAM clock
    gate at 2.4GHz so the real stream starts warm,
  * fp16 gate math on DVE (2x mode, SBUF-only operands), activations
    read PSUM f32 and write fp16; outputs store per work item from
    GpSimd's trigger queue, and the last 512 columns are split 384+128
    on disjoint PSUM regions so the serial ACT+DVE+store tail after the
    final matmul is short.
"""

import os
import sys

for _p in ("/opt/trn_rl_repo", "/root/.axon_site/_ro/trn_rl_repo"):
    if os.path.isdir(_p) and _p not in sys.path:
        sys.path.append(_p)

import numpy as np

B = 65536
D = 256
N_CORES = 8
S = B // N_CORES  # batch rows per core
CH = 512  # batch columns per PSUM bank / compute sub-chunk

# Input-tile load plan: (col_start, width). The first is narrow (and split
# per block) so the pipeline head fills fast; the rest are wide packed
# loads for DMA efficiency.
PLAN = [(0, 512), (512, 512), (1024, 512), (1536, 512)] + [
    (2048 + 1024 * i, 1024) for i in range(6)
]
# block order inside the packed input tensor
_BLOCKS = ("x0", "x1", "h0", "h1")
# matrix order inside the packed weight tensor
_WORDER = ("Wr", "Ur", "Wz", "Uz", "Wh", "Uh")
_BORDER = ("br", "bz", "bh")


def _sub_to_load(j):
    """Map 512-wide sub-chunk j to (load_index, local col offset)."""
    c0 = j * CH
    for li, (start, width) in enumerate(PLAN):
        if start <= c0 < start + width:
            return li, c0 - start
    raise ValueError(j)


def build_nc(s=S, mm_dtype_name=None):
    """Build + compile the per-core Bass program for a shard of s rows."""
    import concourse.bass as bass
    import concourse.mybir as mybir
    import concourse.tile as tile
    from concourse import bacc

    f32 = mybir.dt.float32
    if mm_dtype_name is None:
        mm_dtype_name = os.environ.get("GRU_MM_DTYPE", "float16")
    f16 = getattr(mybir.dt, mm_dtype_name)
    AF = mybir.ActivationFunctionType

    nc = bacc.Bacc("TRN2", target_bir_lowering=False)
    xh = nc.dram_tensor("xh", [128, 4, s], f16, kind="ExternalInput")
    wcat = nc.dram_tensor("wcat", [D, 6 * D], f16, kind="ExternalInput")
    bcat = nc.dram_tensor("bcat", [128, 6], f32, kind="ExternalInput")
    outT = nc.dram_tensor("outT", [D, s], f16, kind="ExternalOutput")

    nsub = s // CH

    with tile.TileContext(nc) as tc:
        with (
            tc.tile_pool(name="const", bufs=1) as cpool,
            tc.tile_pool(name="work", bufs=2) as wpool,
            tc.tile_pool(name="outb", bufs=4) as opool,
            tc.tile_pool(name="psum", bufs=1, space=bass.MemorySpace.PSUM) as ppool,
        ):
            inp = {}  # (block, load_idx) -> AP [128, width]

            # PE warm-up: the HAM clock gate needs ~3.4us of sustained PE
            # activity to lift the engine from 1.2 to 2.4 GHz.  The PE is
            # idle during the head DMAs anyway, so burn that window on
            # dummy matmuls over a memset tile (results go to a scratch
            # PSUM bank that is read once and discarded).
            zt0 = cpool.tile([128, CH], f16, tag="warm", name="warm")
            nc.gpsimd.memset(zt0[:], 0)
            pw = ppool.tile([128, CH], f32, tag="pwarm", name="pwarm")
            for _ in range(10):
                nc.tensor.matmul(pw[:], zt0[:, 0:128], zt0[:], start=True, stop=True)
            wsink = cpool.tile([128, CH], f32, tag="wsink", name="wsink")
            nc.vector.tensor_copy(wsink[:], pw[:])

            # Head DMA scheduling.  Only ~4 DMA-completion semaphores exist
            # per trigger engine, and a trigger past the 4th BLOCKS its
            # engine queue until an earlier DMA fully completes.  So the
            # first four DMAs per engine must be exactly the critical set
            # (r weights + first input tiles), bulk streaming loads go to
            # the otherwise-idle GpSimd queue, and ScalarE's queue frees up
            # early so activations are never stuck behind a trigger.
            wA, wB = {}, {}
            for k in range(2):
                wA[k] = cpool.tile([128, 2 * D], f16, tag=f"wA{k}", name=f"wA{k}")
                wB[k] = cpool.tile([128, 4 * D], f16, tag=f"wB{k}", name=f"wB{k}")

            def load_block(blk, li, eng):
                bi = _BLOCKS.index(blk)
                start, width = PLAN[li]
                t = cpool.tile([128, width], f16, tag=f"i{blk}_{li}",
                               name=f"i{blk}_{li}")
                eng.dma_start(t[:], xh[:, bi, start : start + width])
                inp[(blk, li)] = t

            # wave 1 (the 4 immediate slots per engine): r-gate weights and
            # the j=0 input tiles, i.e. exactly what the first matmuls
            # consume, plus the tiny bias vector
            # x0 takes SyncE's first slot and wA k0 rides GpSimd (which
            # fires immediately on its own semaphore pool), so the two
            # transfers that gate the first matmul both start at the
            # earliest possible trigger tick.  Whole [128,512] tiles: 1KB
            # per-partition lines are the DMA efficiency knee; splitting
            # below that fragments packets.
            load_block("x0", 0, nc.sync)
            nc.gpsimd.dma_start(wA[0][:], wcat[0:128, 0 : 2 * D])
            nc.scalar.dma_start(wA[1][:], wcat[128:256, 0 : 2 * D])
            load_block("x1", 0, nc.scalar)
            # h tiles are needed ~1us after the x tiles (W-before-U order)
            load_block("h0", 0, nc.gpsimd)
            load_block("h1", 0, nc.gpsimd)
            b_sb = cpool.tile([128, 6], f32, tag="bcat")
            nc.sync.dma_start(b_sb[:], bcat[:, :])
            nc.scalar.dma_start(wB[0][:], wcat[0:128, 2 * D : 6 * D])
            # rotation-paced stragglers (each issues once an earlier DMA on
            # its engine completes — this throttling keeps the bulk stream
            # from starving the critical pieces above)
            nc.scalar.dma_start(wB[1][:], wcat[128:256, 2 * D : 6 * D])
            load_block("x1", 1, nc.scalar)
            load_block("h1", 1, nc.scalar)
            load_block("x0", 1, nc.sync)
            load_block("h0", 1, nc.sync)
            for li in range(2, len(PLAN)):
                start, width = PLAN[li]
                t = cpool.tile([128, 4, width], f16, tag=f"ixh_{li}",
                               name=f"ixh_{li}")
                nc.sync.dma_start(t[:], xh[:, :, start : start + width])
                for bi, blk in enumerate(_BLOCKS):
                    inp[(blk, li)] = t[:, bi, :]

            def wap(i, k, g):
                """Weight AP [128,128] for matrix index i (order _WORDER),
                contraction half k, output-feature half g."""
                if i < 2:
                    return wA[k][:, i * D + g * 128 : i * D + (g + 1) * 128]
                return wB[k][:, (i - 2) * D + g * 128 : (i - 2) * D + (g + 1) * 128]

            # Work items: (dram col start, width, psum col offset).  The
            # last 512-wide sub-chunk is split into two 256-wide halves on
            # disjoint PSUM regions so the serial ACT+DVE+store tail after
            # the final matmul covers half as many columns.
            WI = [(j * CH, CH, 0) for j in range(nsub - 1)]
            WI.append(((nsub - 1) * CH, CH // 2, 0))
            WI.append(((nsub - 1) * CH + CH // 2, CH // 2, CH // 2))
            nwi = len(WI)

            def operands(i):
                c0, w, _ = WI[i]
                li, off = _sub_to_load(c0 // CH)
                off += c0 - (c0 // CH) * CH
                sl = slice(off, off + w)
                xs = [inp[(f"x{k}", li)][:, sl] for k in range(2)]
                hs = [inp[(f"h{k}", li)][:, sl] for k in range(2)]
                return xs, hs

            def gate_pair(tagbase, wi, ui, xs, rhs_u, po, w):
                """Both g-halves of one gate.  The W (x-side) matmuls of
                both halves run before the U matmuls: x tiles arrive from
                HBM ~1us before h tiles, and for the candidate gate this
                gives the r*h producer extra slack."""
                ps = []
                for g in range(2):
                    p = ppool.tile([128, CH], f32, tag=f"{tagbase}{g}",
                                   name=f"{tagbase}{g}")
                    ps.append(p)
                for g in range(2):
                    nc.tensor.matmul(ps[g][:, po : po + w], wap(wi, 0, g), xs[0],
                                     start=True, stop=False)
                    nc.tensor.matmul(ps[g][:, po : po + w], wap(wi, 1, g), xs[1],
                                     start=False, stop=False)
                for g in range(2):
                    nc.tensor.matmul(ps[g][:, po : po + w], wap(ui, 0, g),
                                     rhs_u[0], start=False, stop=False)
                    nc.tensor.matmul(ps[g][:, po : po + w], wap(ui, 1, g),
                                     rhs_u[1], start=False, stop=True)
                return ps

            def r_gate(i):
                """reset gate -> r*h tiles for work item i."""
                c0, w, po = WI[i]
                xs, hs = operands(i)
                ps = gate_pair("pr", 0, 1, xs, hs, po, w)
                rh = []
                for g in range(2):
                    rt = wpool.tile([128, CH], f16, tag=f"r{g}", name=f"r{g}")
                    nc.scalar.activation(rt[:, 0:w], ps[g][:, po : po + w],
                                         AF.Sigmoid, bias=b_sb[:, g : g + 1])
                    t = wpool.tile([128, CH], f16, tag=f"rh{g}", name=f"rh{g}")
                    nc.vector.tensor_mul(t[:, 0:w], rt[:, 0:w], hs[g])
                    rh.append(t[:, 0:w])
                return rh

            # software pipeline: r-gate one work item ahead of z/candidate.
            # The pipeline starts at i=1 (not i=0) so the first iteration
            # only needs the i=0 input tiles, which arrive first.
            rh_cur = r_gate(0)
            for i in range(nwi):
                c0, w, po = WI[i]
                xs, hs = operands(i)
                if i == 0:
                    rh_next = None
                elif i == 1:
                    rh_cur = r_gate(1)
                    rh_next = r_gate(2) if nwi > 2 else None
                else:
                    rh_next = r_gate(i + 1) if i + 1 < nwi else None

                pz = gate_pair("pz", 2, 3, xs, hs, po, w)
                zt = []
                for g in range(2):
                    t = wpool.tile([128, CH], f16, tag=f"z{g}", name=f"z{g}")
                    nc.scalar.activation(t[:, 0:w], pz[g][:, po : po + w],
                                         AF.Sigmoid, bias=b_sb[:, 2 + g : 3 + g])
                    zt.append(t)

                ph = gate_pair("ph", 4, 5, xs, rh_cur, po, w)
                for g in range(2):
                    hh = wpool.tile([128, CH], f16, tag=f"hh{g}", name=f"hh{g}")
                    nc.scalar.activation(hh[:, 0:w], ph[g][:, po : po + w],
                                         AF.Tanh, bias=b_sb[:, 4 + g : 5 + g])
                    d = wpool.tile([128, CH], f16, tag=f"d{g}", name=f"d{g}")
                    nc.vector.tensor_sub(d[:, 0:w], hs[g], hh[:, 0:w])
                    m = wpool.tile([128, CH], f16, tag=f"m{g}", name=f"m{g}")
                    nc.vector.tensor_mul(m[:, 0:w], zt[g][:, 0:w], d[:, 0:w])
                    o = opool.tile([128, CH], f16, tag=f"o{g}", name=f"o{g}")
                    nc.vector.tensor_add(o[:, 0:w], hh[:, 0:w], m[:, 0:w])
                    orow = outT[g * 128 : (g + 1) * 128, :]
                    # outputs ride GpSimd's trigger queue: it has its own
                    # semaphore pool so stores never rotation-block the
                    # input stream pacing on SyncE.  The last three items'
                    # stores gate the postamble, so they go whole (never
                    # split below 1KB per-partition lines - fragmentation
                    # makes transfers slower) on the by-then-idle Sync and
                    # Scalar queues, whose triggers are ~350ns cheaper.
                    if i >= nwi - 3:
                        eng = nc.scalar if g == 1 else nc.sync
                    else:
                        eng = nc.gpsimd
                    eng.dma_start(orow[:, c0 : c0 + w], o[:, 0:w])
                rh_cur = rh_next

    nc.compile()
    return nc


_NC_CACHE = {}


def _get_nc():
    key = (S, os.environ.get("GRU_MM_DTYPE", "float16"))
    if key not in _NC_CACHE:
        _NC_CACHE[key] = build_nc(S, key[1])
    return _NC_CACHE[key]


def _make_in_maps(inputs):
    f32 = np.float32
    dt16 = {"float16": np.float16}.get(
        os.environ.get("GRU_MM_DTYPE", "float16")
    )
    if dt16 is None:
        import ml_dtypes

        dt16 = ml_dtypes.bfloat16
    x = np.asarray(inputs["x"], f32)
    h = np.asarray(inputs["h_t_1"], f32)
    wcat = np.ascontiguousarray(
        np.concatenate(
            [np.asarray(inputs[n], f32) for n in ("Wr", "Ur", "Wz", "Uz", "Wh", "Uh")],
            axis=1,
        ).astype(dt16)
    )
    bcat = np.ascontiguousarray(
        np.concatenate(
            [np.asarray(inputs[n], f32).reshape(2, 128).T for n in ("br", "bz", "bh")],
            axis=1,
        )
    )
    consts = {"wcat": wcat, "bcat": bcat}
    in_maps = []
    for c in range(N_CORES):
        sl = slice(c * S, (c + 1) * S)
        xT = x[sl].T.astype(dt16)  # [256, S]
        hT = h[sl].T.astype(dt16)
        xh = np.empty((128, 4, S), dt16)
        xh[:, 0] = xT[0:128]
        xh[:, 1] = xT[128:256]
        xh[:, 2] = hT[0:128]
        xh[:, 3] = hT[128:256]
        m = {"xh": np.ascontiguousarray(xh)}
        m.update(consts)
        in_maps.append(m)
    return in_maps


def run(inputs, trace=False):
    """Run on hardware; returns (h_t ndarray, BassKernelResults)."""
    from concourse.bass_utils import run_bass_kernel_spmd

    nc = _get_nc()
    in_maps = _make_in_maps(inputs)
    res = run_bass_kernel_spmd(nc, in_maps, list(range(N_CORES)), trace=trace)
    out = np.empty((B, D), np.float32)
    for c in range(N_CORES):
        out[c * S : (c + 1) * S] = res.results[c]["outT"].T.astype(np.float32)
    return out, res


def kernel(**inputs):
    out, _ = run(inputs, trace=False)
    return (out, out)
